# revision 8
# baseline (speedup 1.0000x reference)
"""BlipAttention (single-head full-C attention) Bass kernel for 8 Trainium2 NeuronCores.

Reference computation (per batch b of 32):
    qkv  = x @ W_qkv + b_qkv          # [1024, 2304]
    q, k, v = split(qkv, 3)           # each [1024, 768]
    S    = (q @ k.T) / sqrt(768)      # [1024, 1024]
    P    = softmax(S, axis=-1)
    out  = (P @ v) @ W_proj + b_proj  # [1024, 768]

Sharding: data-parallel over the batch dim B=32 -> 4 batches per core, no
collectives.

FLOP reduction via associativity (b_qkv == 0 for this problem):
    S   = q k^T = x (W_q W_k^T) x^T         ->  M  := W_q @ W_k^T   (host)
    out = P (v W_proj) = P (x (W_v W_proj)) ->  M2 := W_v @ W_proj  (host)
so q, k, v and the projection matmul are never materialized.  Per batch the
device computes only:
    T1 = M^T x^T                 [768, 1024]   (lhsT=M chunks, rhs=x^T)
    S^T block = x T1             [128k, 512q]  (lhsT=x^T chunks, rhs=T1)
    P^T = exp(scale * S^T)       (unnormalized; scores are ~N(0,1), exp safe)
    vp = x M2  (+ ones columns)  [1024, 770]   (lhsT=x^T chunks, rhs=M2)
    out block = P vp             (lhsT=P^T chunks, rhs=vp) -- the ones columns
                                 of vp make column 768 the softmax denom
    out = out * (1/denom)        (DVE reciprocal + per-partition scalar mul)
This is 172k PE cycles/batch vs 254k for the direct formulation.

All matmul operands are bf16 (exact product, FP32 accumulate) which runs at
full PE rate; measured end-to-end relative error is ~6e-3 vs the 2e-2 gate
(each quantized tensor contributes ~2-3e-3, adding in quadrature).  bf16
halves DMA bytes and SBUF vs float32r at identical matmul throughput, which
shrinks the cold start (first matmuls wait on M1/x DMA) and on-chip
copy/activation time.  fp8 was evaluated and rejected: quantizing even a
single matmul path to e4m3 gives 4e-2..1.2e-1 relative error.  b_proj is
added on the host after the gather (exact).  The reference's setup_inputs
always produces b_qkv == 0; if a caller ever passes a nonzero b_qkv the
kernel falls back to an exact host computation (the associativity trick
needs the bias folded differently).
"""

import numpy as np

B = 32
SEQ = 1024
C = 768
NCORES = 8
BL = B // NCORES  # batches per core
P = 128
CK = C // P  # 6 contraction chunks of the 768 dim
NK = SEQ // P  # 8 chunks of the sequence dim
NQS = 512  # nq slice width (PSUM free-dim limit for fp32)
NSL = SEQ // NQS  # 2 nq slices
CS = 384  # cout slice width (768 = 2 x 384)
VPW = 772  # vp tile width: 768 data + 2 ones + 2 pad
SCALE = 1.0 / float(np.sqrt(C))

_CACHE = {}


def _build_program(cin):
    """Emit the Bass/Tile program (cin = contraction size, always 768)."""
    import concourse.tile as tile
    import concourse.mybir as mybir
    from concourse import bacc

    F32 = mybir.dt.float32
    BF16 = mybir.dt.bfloat16
    EXP = mybir.ActivationFunctionType.Exp
    ck1 = cin // P

    nc = bacc.Bacc("TRN2", target_bir_lowering=False, debug=False,
                   num_devices=NCORES)
    xT_d = nc.dram_tensor("xT", [BL, cin, SEQ], BF16,
                          kind="ExternalInput").ap()
    m1_d = nc.dram_tensor("m1", [cin, C], BF16, kind="ExternalInput").ap()
    m2_d = nc.dram_tensor("m2", [cin, C], BF16, kind="ExternalInput").ap()
    out_d = nc.dram_tensor("out", [BL, SEQ, C], F32, kind="ExternalOutput").ap()

    with tile.TileContext(nc) as tc:
        with (
            tc.tile_pool(name="consts", bufs=1) as consts,
            tc.tile_pool(name="xtp", bufs=2) as xtp,
            tc.tile_pool(name="t1p", bufs=1) as t1p,
            tc.tile_pool(name="vpp", bufs=1) as vpp,
            tc.tile_pool(name="ptp", bufs=2) as ptp,
            tc.tile_pool(name="rcp", bufs=4) as rcp,
            tc.tile_pool(name="obp", bufs=6) as obp,
            tc.tile_pool(name="mmp", bufs=8, space="PSUM") as mmp,
        ):
            def load_xt(b):
                t = xt_tiles[b]
                for s in range(NSL):
                    for o in range(ck1):
                        nc.sync.dma_start(
                            t[:, o, s * NQS:(s + 1) * NQS],
                            xT_d[b, o * P:(o + 1) * P,
                                 s * NQS:(s + 1) * NQS])

            # Cold start: M1 chunks + x(0) feed the first T1 matmuls; M2 can
            # land later (vp stage follows T1).
            m1 = consts.tile([P, ck1, C], BF16, tag="m1", name="m1")
            m2 = consts.tile([P, ck1, C], BF16, tag="m2", name="m2")
            xt_tiles = {0: xtp.tile([P, ck1, SEQ], BF16, tag="xt", name="xt")}
            # interleave m1 + x(0) chunk-wise to match T1's i-outer
            # consumption order: round i needs only chunk i of each
            xt0 = xt_tiles[0]
            for o in range(ck1):
                nc.sync.dma_start(m1[:, o, :], m1_d[o * P:(o + 1) * P, :])
                for s in range(NSL):
                    nc.sync.dma_start(
                        xt0[:, o, s * NQS:(s + 1) * NQS],
                        xT_d[0, o * P:(o + 1) * P, s * NQS:(s + 1) * NQS])
            for o in range(ck1):
                nc.sync.dma_start(m2[:, o, :], m2_d[o * P:(o + 1) * P, :])
            ones_f = consts.tile([P, 2 * NK], F32, tag="ones_f", name="ones_f")
            nc.vector.memset(ones_f[:], 1.0)

            for b in range(BL):
                if b + 1 < BL:  # prefetch next batch behind this batch's PE work
                    xt_tiles[b + 1] = xtp.tile([P, ck1, SEQ], BF16, tag="xt",
                                               name="xt")
                    load_xt(b + 1)
                xt = xt_tiles[b]

                # T1 = M^T x^T  [768, 1024] : 6 row blocks x 2 seq slices.
                # Contraction (i) is the OUTER loop over 6 concurrent PSUM
                # groups so batch 0's first rounds can start as soon as chunk
                # i of m1/x lands (DMA per chunk ~= PE per round ~= 1.3us).
                t1 = t1p.tile([P, ck1, SEQ], BF16, tag="t1", name="t1")
                for half in range(2):
                    grp = [(blk, s) for s in range(NSL)
                           for blk in range(3 * half, 3 * half + 3)]
                    pss = {g: mmp.tile([P, NQS], F32, tag="mm", name="ps_t")
                           for g in grp}
                    for i in range(ck1):
                        for (blk, s) in grp:
                            nc.tensor.matmul(
                                pss[(blk, s)][:],
                                m1[:, i, blk * P:(blk + 1) * P],
                                xt[:, i, s * NQS:(s + 1) * NQS],
                                start=(i == 0), stop=(i == ck1 - 1))
                    for (blk, s) in grp:
                        nc.vector.tensor_copy(
                            t1[:, blk, s * NQS:(s + 1) * NQS],
                            pss[(blk, s)][:])

                # vp = x M2  [1024 keys, 768] + ones columns at 768/769
                vp = vpp.tile([P, NK, VPW], BF16, tag="vp", name="vp")
                for j in range(NK):
                    nc.scalar.copy(vp[:, j, C:C + 2],
                                   ones_f[:, 2 * j:2 * j + 2])
                for r in range(NK):
                    for cs in range(2):
                        ps = mmp.tile([P, NQS], F32, tag="mm", name="ps_v")
                        for i in range(ck1):
                            nc.tensor.matmul(
                                ps[:, :CS],
                                xt[:, i, r * P:(r + 1) * P],
                                m2[:, i, cs * CS:(cs + 1) * CS],
                                start=(i == 0), stop=(i == ck1 - 1))
                        nc.scalar.copy(vp[:, r, cs * CS:(cs + 1) * CS],
                                       ps[:, :CS])

                for s in range(NSL):
                    nq0 = s * NQS
                    # S^T chunk [128 keys, 512 queries] = x T1, then exp
                    pt = ptp.tile([P, NK, NQS], BF16, tag="pt", name="pt")
                    for j in range(NK):
                        ps = mmp.tile([P, NQS], F32, tag="mm", name="ps_s")
                        for i in range(ck1):
                            nc.tensor.matmul(
                                ps[:],
                                xt[:, i, j * P:(j + 1) * P],
                                t1[:, i, nq0:nq0 + NQS],
                                start=(i == 0), stop=(i == ck1 - 1))
                        nc.scalar.activation(pt[:, j, :], ps[:], EXP,
                                             scale=SCALE)
                    # out block [128 rows, 384 cols] = P vp ; vp's ones columns
                    # ride along in the cs=1 group as the softmax denominator
                    for mi in range(NQS // P):
                        ps1 = mmp.tile([P, NQS], F32, tag="mm", name="ps_o1")
                        for j in range(NK):
                            nc.tensor.matmul(
                                ps1[:, :CS + 2],
                                pt[:, j, mi * P:(mi + 1) * P],
                                vp[:, j, CS:C + 2],
                                start=(j == 0), stop=(j == NK - 1))
                        rc = rcp.tile([P, 1], F32, tag="rc", name="rc")
                        nc.vector.reciprocal(rc[:], ps1[:, CS:CS + 1])
                        ps0 = mmp.tile([P, NQS], F32, tag="mm", name="ps_o0")
                        for j in range(NK):
                            nc.tensor.matmul(
                                ps0[:, :CS],
                                pt[:, j, mi * P:(mi + 1) * P],
                                vp[:, j, 0:CS],
                                start=(j == 0), stop=(j == NK - 1))
                        ob1 = obp.tile([P, CS], F32, tag="ob", name="ob1")
                        nc.vector.tensor_scalar_mul(ob1[:], ps1[:, :CS],
                                                    rc[:, 0:1])
                        nc.sync.dma_start(
                            out_d[b, nq0 + mi * P:nq0 + (mi + 1) * P,
                                  CS:2 * CS], ob1[:])
                        ob0 = obp.tile([P, CS], F32, tag="ob", name="ob0")
                        nc.vector.tensor_scalar_mul(ob0[:], ps0[:, :CS],
                                                    rc[:, 0:1])
                        nc.sync.dma_start(
                            out_d[b, nq0 + mi * P:nq0 + (mi + 1) * P,
                                  0:CS], ob0[:])
    nc.compile()
    return nc


def _get_program(cin):
    if cin not in _CACHE:
        _CACHE[cin] = _build_program(cin)
    return _CACHE[cin]


def _host_reference(x, W_qkv, b_qkv, W_proj, b_proj):
    out = np.empty((B, SEQ, C), dtype=np.float32)
    for b in range(B):
        qkv = x[b] @ W_qkv + b_qkv
        q, k, v = qkv[:, :C], qkv[:, C:2 * C], qkv[:, 2 * C:]
        s = (q @ k.T) * SCALE
        s -= s.max(axis=-1, keepdims=True)
        np.exp(s, out=s)
        s /= s.sum(axis=-1, keepdims=True)
        out[b] = (s @ v) @ W_proj + b_proj
    return out


def run_sharded(x, W_qkv, b_qkv, b_proj, W_proj, trace=False):
    import ml_dtypes
    from concourse.bass_utils import run_bass_kernel_spmd

    x = np.ascontiguousarray(x, dtype=np.float32)
    W_qkv = np.ascontiguousarray(W_qkv, dtype=np.float32)
    W_proj = np.ascontiguousarray(W_proj, dtype=np.float32)
    b_qkv = np.asarray(b_qkv, dtype=np.float32)
    b_proj = np.asarray(b_proj, dtype=np.float32)

    if np.any(b_qkv):
        # Cannot occur for the reference's setup_inputs (b_qkv is zeros);
        # the W_q W_k^T folding assumes zero qkv bias.
        return _host_reference(x, W_qkv, b_qkv, W_proj, b_proj), None

    bf16 = ml_dtypes.bfloat16
    M1 = np.ascontiguousarray(
        (W_qkv[:, :C] @ W_qkv[:, C:2 * C].T).astype(bf16))
    M2 = np.ascontiguousarray((W_qkv[:, 2 * C:] @ W_proj).astype(bf16))
    xT = np.ascontiguousarray(x.transpose(0, 2, 1).astype(bf16))  # [B,C,SEQ]
    nc = _get_program(C)
    in_maps = [
        {"xT": xT[c * BL:(c + 1) * BL], "m1": M1, "m2": M2}
        for c in range(NCORES)
    ]
    res = run_bass_kernel_spmd(nc, in_maps, core_ids=list(range(NCORES)),
                               trace=trace)
    out = np.concatenate([res.results[c]["out"] for c in range(NCORES)],
                         axis=0)
    out = out + b_proj[None, None, :]
    return out.astype(np.float32), res


def kernel(x, W_qkv, b_qkv, W_proj, b_proj):
    out, _ = run_sharded(x, W_qkv, b_qkv, b_proj, W_proj, trace=False)
    return out


# revision 10
# speedup vs baseline: 1.1279x; 1.1279x over previous
"""BlipAttention (single-head full-C attention) Bass kernel for 8 Trainium2 NeuronCores.

Reference computation (per batch b of 32):
    qkv  = x @ W_qkv + b_qkv          # [1024, 2304]
    q, k, v = split(qkv, 3)           # each [1024, 768]
    S    = (q @ k.T) / sqrt(768)      # [1024, 1024]
    P    = softmax(S, axis=-1)
    out  = (P @ v) @ W_proj + b_proj  # [1024, 768]

Sharding: data-parallel over the batch dim B=32 -> 4 batches per core, no
collectives.

FLOP reduction via associativity (b_qkv == 0 for this problem):
    S   = q k^T = x (W_q W_k^T) x^T         ->  M  := W_q @ W_k^T   (host)
    out = P (v W_proj) = P (x (W_v W_proj)) ->  M2 := W_v @ W_proj  (host)
so q, k, v and the projection matmul are never materialized.  Per batch the
device computes only:
    T1 = M^T x^T                 [768, 1024]   (lhsT=M chunks, rhs=x^T)
    S^T block = x T1             [128k, 512q]  (lhsT=x^T chunks, rhs=T1)
    P^T = exp(scale * S^T)       (unnormalized; scores are ~N(0,1), exp safe)
    vp = x M2  (+ ones columns)  [1024, 770]   (lhsT=x^T chunks, rhs=M2)
    out block = P vp             (lhsT=P^T chunks, rhs=vp) -- the ones columns
                                 of vp make column 768 the softmax denom
    out = out * (1/denom)        (DVE reciprocal + per-partition scalar mul)
This is 172k PE cycles/batch vs 254k for the direct formulation.

All matmul operands are bf16 (exact product, FP32 accumulate) which runs at
full PE rate; measured end-to-end relative error is ~6e-3 vs the 2e-2 gate
(each quantized tensor contributes ~2-3e-3, adding in quadrature).  bf16
halves DMA bytes and SBUF vs float32r at identical matmul throughput, which
shrinks the cold start (first matmuls wait on M1/x DMA) and on-chip
copy/activation time.  fp8 was evaluated and rejected: quantizing even a
single matmul path to e4m3 gives 4e-2..1.2e-1 relative error.  b_proj is
added on the host after the gather (exact).  The reference's setup_inputs
always produces b_qkv == 0; if a caller ever passes a nonzero b_qkv the
kernel falls back to an exact host computation (the associativity trick
needs the bias folded differently).
"""

import numpy as np

B = 32
SEQ = 1024
C = 768
NCORES = 8
BL = B // NCORES  # batches per core
P = 128
CK = C // P  # 6 contraction chunks of the 768 dim
NK = SEQ // P  # 8 chunks of the sequence dim
NQS = 512  # nq slice width (PSUM free-dim limit for fp32)
NSL = SEQ // NQS  # 2 nq slices
CS = 384  # cout slice width (768 = 2 x 384)
VPW = 772  # vp tile width: 768 data + 2 ones + 2 pad
SCALE = 1.0 / float(np.sqrt(C))

_CACHE = {}


def _build_program(cin):
    """Emit the Bass/Tile program (cin = contraction size, always 768)."""
    import concourse.tile as tile
    import concourse.mybir as mybir
    from concourse import bacc

    F32 = mybir.dt.float32
    BF16 = mybir.dt.bfloat16
    EXP = mybir.ActivationFunctionType.Exp
    ck1 = cin // P

    nc = bacc.Bacc("TRN2", target_bir_lowering=False, debug=False,
                   num_devices=NCORES)
    xT_d = nc.dram_tensor("xT", [BL, cin, SEQ], BF16,
                          kind="ExternalInput").ap()
    m1_d = nc.dram_tensor("m1", [cin, C], BF16, kind="ExternalInput").ap()
    m2_d = nc.dram_tensor("m2", [cin, C], BF16, kind="ExternalInput").ap()
    out_d = nc.dram_tensor("out", [BL, SEQ, C], F32, kind="ExternalOutput").ap()

    with tile.TileContext(nc) as tc:
        with (
            tc.tile_pool(name="consts", bufs=1) as consts,
            tc.tile_pool(name="xtp", bufs=2) as xtp,
            tc.tile_pool(name="t1p", bufs=1) as t1p,
            tc.tile_pool(name="vpp", bufs=1) as vpp,
            tc.tile_pool(name="ptp", bufs=2) as ptp,
            tc.tile_pool(name="rcp", bufs=4) as rcp,
            tc.tile_pool(name="obp", bufs=6) as obp,
            tc.tile_pool(name="mmp", bufs=8, space="PSUM") as mmp,
        ):
            def load_xt(b):
                t = xt_tiles[b]
                for s in range(NSL):
                    for o in range(ck1):
                        nc.sync.dma_start(
                            t[:, o, s * NQS:(s + 1) * NQS],
                            xT_d[b, o * P:(o + 1) * P,
                                 s * NQS:(s + 1) * NQS])

            # Cold start: M1 chunks + x(0) feed the first T1 matmuls; M2 can
            # land later (vp stage follows T1).
            m1 = consts.tile([P, ck1, C], BF16, tag="m1", name="m1")
            m2 = consts.tile([P, ck1, C], BF16, tag="m2", name="m2")
            xt_tiles = {0: xtp.tile([P, ck1, SEQ], BF16, tag="xt", name="xt")}
            # Cold-start DMA order matches T1(0)'s (s, blk) consumption:
            # group (s0, blk0) needs only m1 column-block 0 + the s0 half of
            # x(0) (~1MB, ~2.7us), later blocks land while the PE computes.
            xt0 = xt_tiles[0]

            def load_m1_colblk(blk):
                for o in range(ck1):
                    nc.sync.dma_start(
                        m1[:, o, blk * P:(blk + 1) * P],
                        m1_d[o * P:(o + 1) * P, blk * P:(blk + 1) * P])

            load_m1_colblk(0)
            for o in range(ck1):
                nc.sync.dma_start(xt0[:, o, 0:NQS],
                                  xT_d[0, o * P:(o + 1) * P, 0:NQS])
            for blk in range(1, CK):
                load_m1_colblk(blk)
            for o in range(ck1):
                nc.sync.dma_start(xt0[:, o, NQS:SEQ],
                                  xT_d[0, o * P:(o + 1) * P, NQS:SEQ])
            for o in range(ck1):
                nc.sync.dma_start(m2[:, o, :], m2_d[o * P:(o + 1) * P, :])
            ones_f = consts.tile([P, 2 * NK], F32, tag="ones_f", name="ones_f")
            nc.vector.memset(ones_f[:], 1.0)

            for b in range(BL):
                if b + 1 < BL:  # prefetch next batch behind this batch's PE work
                    xt_tiles[b + 1] = xtp.tile([P, ck1, SEQ], BF16, tag="xt",
                                               name="xt")
                    load_xt(b + 1)
                xt = xt_tiles[b]

                # T1 = M^T x^T  [768, 1024] : 2 seq slices x 6 row blocks.
                # Accumulation chains stay contiguous on one PSUM bank
                # (interleaving banks mid-chain costs ~200ns/matmul on HW).
                t1 = t1p.tile([P, ck1, SEQ], BF16, tag="t1", name="t1")
                for s in range(NSL):
                    for blk in range(CK):
                        ps = mmp.tile([P, NQS], F32, tag="mm", name="ps_t")
                        for i in range(ck1):
                            nc.tensor.matmul(
                                ps[:],
                                m1[:, i, blk * P:(blk + 1) * P],
                                xt[:, i, s * NQS:(s + 1) * NQS],
                                start=(i == 0), stop=(i == ck1 - 1))
                        nc.vector.tensor_copy(
                            t1[:, blk, s * NQS:(s + 1) * NQS], ps[:])

                # vp = x M2  [1024 keys, 768] + ones columns at 768/769
                vp = vpp.tile([P, NK, VPW], BF16, tag="vp", name="vp")
                for j in range(NK):
                    nc.scalar.copy(vp[:, j, C:C + 2],
                                   ones_f[:, 2 * j:2 * j + 2])
                for r in range(NK):
                    for cs in range(2):
                        ps = mmp.tile([P, NQS], F32, tag="mm", name="ps_v")
                        for i in range(ck1):
                            nc.tensor.matmul(
                                ps[:, :CS],
                                xt[:, i, r * P:(r + 1) * P],
                                m2[:, i, cs * CS:(cs + 1) * CS],
                                start=(i == 0), stop=(i == ck1 - 1))
                        nc.scalar.copy(vp[:, r, cs * CS:(cs + 1) * CS],
                                       ps[:, :CS])

                for s in range(NSL):
                    nq0 = s * NQS
                    # S^T chunk [128 keys, 512 queries] = x T1, then exp
                    pt = ptp.tile([P, NK, NQS], BF16, tag="pt", name="pt")
                    for j in range(NK):
                        ps = mmp.tile([P, NQS], F32, tag="mm", name="ps_s")
                        for i in range(ck1):
                            nc.tensor.matmul(
                                ps[:],
                                xt[:, i, j * P:(j + 1) * P],
                                t1[:, i, nq0:nq0 + NQS],
                                start=(i == 0), stop=(i == ck1 - 1))
                        nc.scalar.activation(pt[:, j, :], ps[:], EXP,
                                             scale=SCALE)
                    # out block [128 rows, 384 cols] = P vp ; vp's ones columns
                    # ride along in the cs=1 group as the softmax denominator
                    for mi in range(NQS // P):
                        ps1 = mmp.tile([P, NQS], F32, tag="mm", name="ps_o1")
                        for j in range(NK):
                            nc.tensor.matmul(
                                ps1[:, :CS + 2],
                                pt[:, j, mi * P:(mi + 1) * P],
                                vp[:, j, CS:C + 2],
                                start=(j == 0), stop=(j == NK - 1))
                        rc = rcp.tile([P, 1], F32, tag="rc", name="rc")
                        nc.vector.reciprocal(rc[:], ps1[:, CS:CS + 1])
                        ps0 = mmp.tile([P, NQS], F32, tag="mm", name="ps_o0")
                        for j in range(NK):
                            nc.tensor.matmul(
                                ps0[:, :CS],
                                pt[:, j, mi * P:(mi + 1) * P],
                                vp[:, j, 0:CS],
                                start=(j == 0), stop=(j == NK - 1))
                        ob1 = obp.tile([P, CS], F32, tag="ob", name="ob1")
                        nc.vector.tensor_scalar_mul(ob1[:], ps1[:, :CS],
                                                    rc[:, 0:1])
                        nc.sync.dma_start(
                            out_d[b, nq0 + mi * P:nq0 + (mi + 1) * P,
                                  CS:2 * CS], ob1[:])
                        ob0 = obp.tile([P, CS], F32, tag="ob", name="ob0")
                        nc.vector.tensor_scalar_mul(ob0[:], ps0[:, :CS],
                                                    rc[:, 0:1])
                        nc.sync.dma_start(
                            out_d[b, nq0 + mi * P:nq0 + (mi + 1) * P,
                                  0:CS], ob0[:])
    nc.compile()
    return nc


def _get_program(cin):
    if cin not in _CACHE:
        _CACHE[cin] = _build_program(cin)
    return _CACHE[cin]


def _host_reference(x, W_qkv, b_qkv, W_proj, b_proj):
    out = np.empty((B, SEQ, C), dtype=np.float32)
    for b in range(B):
        qkv = x[b] @ W_qkv + b_qkv
        q, k, v = qkv[:, :C], qkv[:, C:2 * C], qkv[:, 2 * C:]
        s = (q @ k.T) * SCALE
        s -= s.max(axis=-1, keepdims=True)
        np.exp(s, out=s)
        s /= s.sum(axis=-1, keepdims=True)
        out[b] = (s @ v) @ W_proj + b_proj
    return out


def run_sharded(x, W_qkv, b_qkv, b_proj, W_proj, trace=False):
    import ml_dtypes
    from concourse.bass_utils import run_bass_kernel_spmd

    x = np.ascontiguousarray(x, dtype=np.float32)
    W_qkv = np.ascontiguousarray(W_qkv, dtype=np.float32)
    W_proj = np.ascontiguousarray(W_proj, dtype=np.float32)
    b_qkv = np.asarray(b_qkv, dtype=np.float32)
    b_proj = np.asarray(b_proj, dtype=np.float32)

    if np.any(b_qkv):
        # Cannot occur for the reference's setup_inputs (b_qkv is zeros);
        # the W_q W_k^T folding assumes zero qkv bias.
        return _host_reference(x, W_qkv, b_qkv, W_proj, b_proj), None

    bf16 = ml_dtypes.bfloat16
    M1 = np.ascontiguousarray(
        (W_qkv[:, :C] @ W_qkv[:, C:2 * C].T).astype(bf16))
    M2 = np.ascontiguousarray((W_qkv[:, 2 * C:] @ W_proj).astype(bf16))
    xT = np.ascontiguousarray(x.transpose(0, 2, 1).astype(bf16))  # [B,C,SEQ]
    nc = _get_program(C)
    in_maps = [
        {"xT": xT[c * BL:(c + 1) * BL], "m1": M1, "m2": M2}
        for c in range(NCORES)
    ]
    res = run_bass_kernel_spmd(nc, in_maps, core_ids=list(range(NCORES)),
                               trace=trace)
    out = np.concatenate([res.results[c]["out"] for c in range(NCORES)],
                         axis=0)
    out = out + b_proj[None, None, :]
    return out.astype(np.float32), res


def kernel(x, W_qkv, b_qkv, W_proj, b_proj):
    out, _ = run_sharded(x, W_qkv, b_qkv, b_proj, W_proj, trace=False)
    return out


# revision 11
# speedup vs baseline: 1.1728x; 1.0398x over previous
"""BlipAttention (single-head full-C attention) Bass kernel for 8 Trainium2 NeuronCores.

Reference computation (per batch b of 32):
    qkv  = x @ W_qkv + b_qkv          # [1024, 2304]
    q, k, v = split(qkv, 3)           # each [1024, 768]
    S    = (q @ k.T) / sqrt(768)      # [1024, 1024]
    P    = softmax(S, axis=-1)
    out  = (P @ v) @ W_proj + b_proj  # [1024, 768]

Sharding: data-parallel over the batch dim B=32 -> 4 batches per core, no
collectives.

FLOP reduction via associativity (b_qkv == 0 for this problem):
    S   = q k^T = x (W_q W_k^T) x^T         ->  M  := W_q @ W_k^T   (host)
    out = P (v W_proj) = P (x (W_v W_proj)) ->  M2 := W_v @ W_proj  (host)
so q, k, v and the projection matmul are never materialized.  Per batch the
device computes only:
    T1 = M^T x^T                 [768, 1024]   (lhsT=M chunks, rhs=x^T)
    S^T block = x T1             [128k, 512q]  (lhsT=x^T chunks, rhs=T1)
    P^T = exp(scale * S^T)       (unnormalized; scores are ~N(0,1), exp safe)
    vp = x M2  (+ ones columns)  [1024, 770]   (lhsT=x^T chunks, rhs=M2)
    out block = P vp             (lhsT=P^T chunks, rhs=vp) -- the ones columns
                                 of vp make column 768 the softmax denom
    out = out * (1/denom)        (DVE reciprocal + per-partition scalar mul)
This is 172k PE cycles/batch vs 254k for the direct formulation.

All matmul operands are bf16 (exact product, FP32 accumulate) which runs at
full PE rate; measured end-to-end relative error is ~6e-3 vs the 2e-2 gate
(each quantized tensor contributes ~2-3e-3, adding in quadrature).  bf16
halves DMA bytes and SBUF vs float32r at identical matmul throughput, which
shrinks the cold start (first matmuls wait on M1/x DMA) and on-chip
copy/activation time.  fp8 was evaluated and rejected: quantizing even a
single matmul path to e4m3 gives 4e-2..1.2e-1 relative error.  b_proj is
added on the host after the gather (exact).  The reference's setup_inputs
always produces b_qkv == 0; if a caller ever passes a nonzero b_qkv the
kernel falls back to an exact host computation (the associativity trick
needs the bias folded differently).
"""

import numpy as np

B = 32
SEQ = 1024
C = 768
NCORES = 8
BL = B // NCORES  # batches per core
P = 128
CK = C // P  # 6 contraction chunks of the 768 dim
NK = SEQ // P  # 8 chunks of the sequence dim
NQS = 512  # nq slice width (PSUM free-dim limit for fp32)
NSL = SEQ // NQS  # 2 nq slices
CS = 384  # cout slice width (768 = 2 x 384)
VPW = 772  # vp tile width: 768 data + 2 ones + 2 pad
SCALE = 1.0 / float(np.sqrt(C))

_CACHE = {}


def _build_program(cin):
    """Emit the Bass/Tile program (cin = contraction size, always 768)."""
    import concourse.tile as tile
    import concourse.mybir as mybir
    from concourse import bacc

    F32 = mybir.dt.float32
    BF16 = mybir.dt.bfloat16
    EXP = mybir.ActivationFunctionType.Exp
    ck1 = cin // P

    nc = bacc.Bacc("TRN2", target_bir_lowering=False, debug=False,
                   num_devices=NCORES)
    xT_d = nc.dram_tensor("xT", [BL, cin, SEQ], BF16,
                          kind="ExternalInput").ap()
    m1_d = nc.dram_tensor("m1", [cin, C], BF16, kind="ExternalInput").ap()
    m2_d = nc.dram_tensor("m2", [cin, C], BF16, kind="ExternalInput").ap()
    out_d = nc.dram_tensor("out", [BL, SEQ, C], F32, kind="ExternalOutput").ap()

    with tile.TileContext(nc) as tc:
        with (
            tc.tile_pool(name="consts", bufs=1) as consts,
            tc.tile_pool(name="xtp", bufs=2) as xtp,
            tc.tile_pool(name="t1p", bufs=1) as t1p,
            tc.tile_pool(name="vpp", bufs=1) as vpp,
            tc.tile_pool(name="ptp", bufs=2) as ptp,
            tc.tile_pool(name="rcp", bufs=4) as rcp,
            tc.tile_pool(name="obp", bufs=6) as obp,
            tc.tile_pool(name="mmp", bufs=8, space="PSUM") as mmp,
        ):
            def load_xt(b):
                t = xt_tiles[b]
                for s in range(NSL):
                    for o in range(ck1):
                        nc.sync.dma_start(
                            t[:, o, s * NQS:(s + 1) * NQS],
                            xT_d[b, o * P:(o + 1) * P,
                                 s * NQS:(s + 1) * NQS])

            # Cold start: M1 chunks + x(0) feed the first T1 matmuls; M2 can
            # land later (vp stage follows T1).
            m1 = consts.tile([P, ck1, C], BF16, tag="m1", name="m1")
            m2 = consts.tile([P, ck1, C], BF16, tag="m2", name="m2")
            xt_tiles = {0: xtp.tile([P, ck1, SEQ], BF16, tag="xt", name="xt")}
            # Cold-start DMA order matches T1(0)'s (s, blk) consumption:
            # m1 + the s0 half of x(0) first (first T1 group can finish at
            # ~5.5us); the s1 half isn't consumed until ~13us.
            xt0 = xt_tiles[0]
            for o in range(ck1):
                nc.sync.dma_start(m1[:, o, :], m1_d[o * P:(o + 1) * P, :])
            for o in range(ck1):
                nc.sync.dma_start(xt0[:, o, 0:NQS],
                                  xT_d[0, o * P:(o + 1) * P, 0:NQS])
            for o in range(ck1):
                nc.sync.dma_start(xt0[:, o, NQS:SEQ],
                                  xT_d[0, o * P:(o + 1) * P, NQS:SEQ])
            for o in range(ck1):
                nc.sync.dma_start(m2[:, o, :], m2_d[o * P:(o + 1) * P, :])
            ones_f = consts.tile([P, 2 * NK], F32, tag="ones_f", name="ones_f")
            nc.vector.memset(ones_f[:], 1.0)

            for b in range(BL):
                if b + 1 < BL:  # prefetch next batch behind this batch's PE work
                    xt_tiles[b + 1] = xtp.tile([P, ck1, SEQ], BF16, tag="xt",
                                               name="xt")
                    load_xt(b + 1)
                xt = xt_tiles[b]

                # T1 = M^T x^T  [768, 1024] : 2 seq slices x 6 row blocks.
                # Accumulation chains stay contiguous on one PSUM bank
                # (interleaving banks mid-chain costs ~200ns/matmul on HW).
                t1 = t1p.tile([P, ck1, SEQ], BF16, tag="t1", name="t1")
                for s in range(NSL):
                    for blk in range(CK):
                        ps = mmp.tile([P, NQS], F32, tag="mm", name="ps_t")
                        for i in range(ck1):
                            nc.tensor.matmul(
                                ps[:],
                                m1[:, i, blk * P:(blk + 1) * P],
                                xt[:, i, s * NQS:(s + 1) * NQS],
                                start=(i == 0), stop=(i == ck1 - 1))
                        nc.vector.tensor_copy(
                            t1[:, blk, s * NQS:(s + 1) * NQS], ps[:])

                # vp = x M2  [1024 keys, 768] + ones columns at 768/769
                vp = vpp.tile([P, NK, VPW], BF16, tag="vp", name="vp")
                for j in range(NK):
                    nc.scalar.copy(vp[:, j, C:C + 2],
                                   ones_f[:, 2 * j:2 * j + 2])
                for r in range(NK):
                    for cs in range(2):
                        ps = mmp.tile([P, NQS], F32, tag="mm", name="ps_v")
                        for i in range(ck1):
                            nc.tensor.matmul(
                                ps[:, :CS],
                                xt[:, i, r * P:(r + 1) * P],
                                m2[:, i, cs * CS:(cs + 1) * CS],
                                start=(i == 0), stop=(i == ck1 - 1))
                        nc.scalar.copy(vp[:, r, cs * CS:(cs + 1) * CS],
                                       ps[:, :CS])

                for s in range(NSL):
                    nq0 = s * NQS
                    # S^T chunk [128 keys, 512 queries] = x T1, then exp
                    pt = ptp.tile([P, NK, NQS], BF16, tag="pt", name="pt")
                    for j in range(NK):
                        ps = mmp.tile([P, NQS], F32, tag="mm", name="ps_s")
                        for i in range(ck1):
                            nc.tensor.matmul(
                                ps[:],
                                xt[:, i, j * P:(j + 1) * P],
                                t1[:, i, nq0:nq0 + NQS],
                                start=(i == 0), stop=(i == ck1 - 1))
                        nc.scalar.activation(pt[:, j, :], ps[:], EXP,
                                             scale=SCALE)
                    # out block [128 rows, 384 cols] = P vp ; vp's ones columns
                    # ride along in the cs=1 group as the softmax denominator
                    for mi in range(NQS // P):
                        ps1 = mmp.tile([P, NQS], F32, tag="mm", name="ps_o1")
                        for j in range(NK):
                            nc.tensor.matmul(
                                ps1[:, :CS + 2],
                                pt[:, j, mi * P:(mi + 1) * P],
                                vp[:, j, CS:C + 2],
                                start=(j == 0), stop=(j == NK - 1))
                        rc = rcp.tile([P, 1], F32, tag="rc", name="rc")
                        nc.vector.reciprocal(rc[:], ps1[:, CS:CS + 1])
                        ps0 = mmp.tile([P, NQS], F32, tag="mm", name="ps_o0")
                        for j in range(NK):
                            nc.tensor.matmul(
                                ps0[:, :CS],
                                pt[:, j, mi * P:(mi + 1) * P],
                                vp[:, j, 0:CS],
                                start=(j == 0), stop=(j == NK - 1))
                        ob1 = obp.tile([P, CS], F32, tag="ob", name="ob1")
                        nc.vector.tensor_scalar_mul(ob1[:], ps1[:, :CS],
                                                    rc[:, 0:1])
                        nc.sync.dma_start(
                            out_d[b, nq0 + mi * P:nq0 + (mi + 1) * P,
                                  CS:2 * CS], ob1[:])
                        ob0 = obp.tile([P, CS], F32, tag="ob", name="ob0")
                        nc.vector.tensor_scalar_mul(ob0[:], ps0[:, :CS],
                                                    rc[:, 0:1])
                        nc.sync.dma_start(
                            out_d[b, nq0 + mi * P:nq0 + (mi + 1) * P,
                                  0:CS], ob0[:])
    nc.compile()
    return nc


def _get_program(cin):
    if cin not in _CACHE:
        _CACHE[cin] = _build_program(cin)
    return _CACHE[cin]


def _host_reference(x, W_qkv, b_qkv, W_proj, b_proj):
    out = np.empty((B, SEQ, C), dtype=np.float32)
    for b in range(B):
        qkv = x[b] @ W_qkv + b_qkv
        q, k, v = qkv[:, :C], qkv[:, C:2 * C], qkv[:, 2 * C:]
        s = (q @ k.T) * SCALE
        s -= s.max(axis=-1, keepdims=True)
        np.exp(s, out=s)
        s /= s.sum(axis=-1, keepdims=True)
        out[b] = (s @ v) @ W_proj + b_proj
    return out


def run_sharded(x, W_qkv, b_qkv, b_proj, W_proj, trace=False):
    import ml_dtypes
    from concourse.bass_utils import run_bass_kernel_spmd

    x = np.ascontiguousarray(x, dtype=np.float32)
    W_qkv = np.ascontiguousarray(W_qkv, dtype=np.float32)
    W_proj = np.ascontiguousarray(W_proj, dtype=np.float32)
    b_qkv = np.asarray(b_qkv, dtype=np.float32)
    b_proj = np.asarray(b_proj, dtype=np.float32)

    if np.any(b_qkv):
        # Cannot occur for the reference's setup_inputs (b_qkv is zeros);
        # the W_q W_k^T folding assumes zero qkv bias.
        return _host_reference(x, W_qkv, b_qkv, W_proj, b_proj), None

    bf16 = ml_dtypes.bfloat16
    M1 = np.ascontiguousarray(
        (W_qkv[:, :C] @ W_qkv[:, C:2 * C].T).astype(bf16))
    M2 = np.ascontiguousarray((W_qkv[:, 2 * C:] @ W_proj).astype(bf16))
    xT = np.ascontiguousarray(x.transpose(0, 2, 1).astype(bf16))  # [B,C,SEQ]
    nc = _get_program(C)
    in_maps = [
        {"xT": xT[c * BL:(c + 1) * BL], "m1": M1, "m2": M2}
        for c in range(NCORES)
    ]
    res = run_bass_kernel_spmd(nc, in_maps, core_ids=list(range(NCORES)),
                               trace=trace)
    out = np.concatenate([res.results[c]["out"] for c in range(NCORES)],
                         axis=0)
    out = out + b_proj[None, None, :]
    return out.astype(np.float32), res


def kernel(x, W_qkv, b_qkv, W_proj, b_proj):
    out, _ = run_sharded(x, W_qkv, b_qkv, b_proj, W_proj, trace=False)
    return out


# revision 13
# speedup vs baseline: 1.1782x; 1.0046x over previous
"""BlipAttention (single-head full-C attention) Bass kernel for 8 Trainium2 NeuronCores.

Reference computation (per batch b of 32):
    qkv  = x @ W_qkv + b_qkv          # [1024, 2304]
    q, k, v = split(qkv, 3)           # each [1024, 768]
    S    = (q @ k.T) / sqrt(768)      # [1024, 1024]
    P    = softmax(S, axis=-1)
    out  = (P @ v) @ W_proj + b_proj  # [1024, 768]

Sharding: data-parallel over the batch dim B=32 -> 4 batches per core, no
collectives.

FLOP reduction via associativity (b_qkv == 0 for this problem):
    S   = q k^T = x (W_q W_k^T) x^T         ->  M  := W_q @ W_k^T   (host)
    out = P (v W_proj) = P (x (W_v W_proj)) ->  M2 := W_v @ W_proj  (host)
so q, k, v and the projection matmul are never materialized.  Per batch the
device computes only:
    T1 = M^T x^T                 [768, 1024]   (lhsT=M chunks, rhs=x^T)
    S^T block = x T1             [128k, 512q]  (lhsT=x^T chunks, rhs=T1)
    P^T = exp(scale * S^T)       (unnormalized; scores are ~N(0,1), exp safe)
    vp = x M2  (+ ones columns)  [1024, 770]   (lhsT=x^T chunks, rhs=M2)
    out block = P vp             (lhsT=P^T chunks, rhs=vp) -- the ones columns
                                 of vp make column 768 the softmax denom
    out = out * (1/denom)        (DVE reciprocal + per-partition scalar mul)
This is 172k PE cycles/batch vs 254k for the direct formulation.

All matmul operands are bf16 (exact product, FP32 accumulate) which runs at
full PE rate; measured end-to-end relative error is ~6e-3 vs the 2e-2 gate
(each quantized tensor contributes ~2-3e-3, adding in quadrature).  bf16
halves DMA bytes and SBUF vs float32r at identical matmul throughput, which
shrinks the cold start (first matmuls wait on M1/x DMA) and on-chip
copy/activation time.  fp8 was evaluated and rejected: quantizing even a
single matmul path to e4m3 gives 4e-2..1.2e-1 relative error.  b_proj is
added on the host after the gather (exact).  The reference's setup_inputs
always produces b_qkv == 0; if a caller ever passes a nonzero b_qkv the
kernel falls back to an exact host computation (the associativity trick
needs the bias folded differently).
"""

import numpy as np

B = 32
SEQ = 1024
C = 768
NCORES = 8
BL = B // NCORES  # batches per core
P = 128
CK = C // P  # 6 contraction chunks of the 768 dim
NK = SEQ // P  # 8 chunks of the sequence dim
NQS = 512  # nq slice width (PSUM free-dim limit for fp32)
NSL = SEQ // NQS  # 2 nq slices
CS = 384  # cout slice width (768 = 2 x 384)
VPW = 772  # vp tile width: 768 data + 2 ones + 2 pad
SCALE = 1.0 / float(np.sqrt(C))

_CACHE = {}


def _build_program(cin):
    """Emit the Bass/Tile program (cin = contraction size, always 768)."""
    import concourse.tile as tile
    import concourse.mybir as mybir
    from concourse import bacc

    F32 = mybir.dt.float32
    BF16 = mybir.dt.bfloat16
    EXP = mybir.ActivationFunctionType.Exp
    ck1 = cin // P

    nc = bacc.Bacc("TRN2", target_bir_lowering=False, debug=False,
                   num_devices=NCORES)
    xT_d = nc.dram_tensor("xT", [BL, cin, SEQ], BF16,
                          kind="ExternalInput").ap()
    m1_d = nc.dram_tensor("m1", [cin, C], BF16, kind="ExternalInput").ap()
    m2_d = nc.dram_tensor("m2", [cin, C], BF16, kind="ExternalInput").ap()
    out_d = nc.dram_tensor("out", [BL, SEQ, C], F32, kind="ExternalOutput").ap()

    with tile.TileContext(nc) as tc:
        with (
            tc.tile_pool(name="consts", bufs=1) as consts,
            tc.tile_pool(name="xtp", bufs=2) as xtp,
            tc.tile_pool(name="t1p", bufs=1) as t1p,
            tc.tile_pool(name="vpp", bufs=1) as vpp,
            tc.tile_pool(name="ptp", bufs=2) as ptp,
            tc.tile_pool(name="rcp", bufs=4) as rcp,
            tc.tile_pool(name="obp", bufs=6) as obp,
            tc.tile_pool(name="mmp", bufs=8, space="PSUM") as mmp,
        ):
            def load_xt(b):
                t = xt_tiles[b]
                for s in range(NSL):
                    for o in range(ck1):
                        nc.sync.dma_start(
                            t[:, o, s * NQS:(s + 1) * NQS],
                            xT_d[b, o * P:(o + 1) * P,
                                 s * NQS:(s + 1) * NQS])

            # Cold start: M1 chunks + x(0) feed the first T1 matmuls; M2 can
            # land later (vp stage follows T1).
            m1 = consts.tile([P, ck1, C], BF16, tag="m1", name="m1")
            m2 = consts.tile([P, ck1, C], BF16, tag="m2", name="m2")
            xt_tiles = {0: xtp.tile([P, ck1, SEQ], BF16, tag="xt", name="xt")}
            # Cold-start DMA order matches T1(0)'s (s, blk) consumption:
            # m1 + the s0 half of x(0) first (first T1 group can finish at
            # ~5.5us); the s1 half isn't consumed until ~13us.
            xt0 = xt_tiles[0]
            for o in range(ck1):
                nc.sync.dma_start(m1[:, o, :], m1_d[o * P:(o + 1) * P, :])
            for o in range(ck1):
                nc.sync.dma_start(xt0[:, o, 0:NQS],
                                  xT_d[0, o * P:(o + 1) * P, 0:NQS])
            for o in range(ck1):
                nc.sync.dma_start(xt0[:, o, NQS:SEQ],
                                  xT_d[0, o * P:(o + 1) * P, NQS:SEQ])
            for o in range(ck1):
                nc.sync.dma_start(m2[:, o, :], m2_d[o * P:(o + 1) * P, :])
            ones_f = consts.tile([P, 2 * NK], F32, tag="ones_f", name="ones_f")
            nc.vector.memset(ones_f[:], 1.0)

            for b in range(BL):
                xt = xt_tiles[b]

                # T1 = M^T x^T  [768, 1024] : 2 seq slices x 6 row blocks.
                # Accumulation chains stay contiguous on one PSUM bank
                # (interleaving banks mid-chain costs ~200ns/matmul on HW).
                t1 = t1p.tile([P, ck1, SEQ], BF16, tag="t1", name="t1")
                for s in range(NSL):
                    for blk in range(CK):
                        ps = mmp.tile([P, NQS], F32, tag="mm", name="ps_t")
                        for i in range(ck1):
                            nc.tensor.matmul(
                                ps[:],
                                m1[:, i, blk * P:(blk + 1) * P],
                                xt[:, i, s * NQS:(s + 1) * NQS],
                                start=(i == 0), stop=(i == ck1 - 1))
                        nc.vector.tensor_copy(
                            t1[:, blk, s * NQS:(s + 1) * NQS], ps[:])

                # Prefetch the next batch only now: DMA descriptors share
                # bandwidth round-robin, so queueing this before T1 starves
                # the cold-start-critical m1/x(0) transfers.
                if b + 1 < BL:
                    xt_tiles[b + 1] = xtp.tile([P, ck1, SEQ], BF16, tag="xt",
                                               name="xt")
                    load_xt(b + 1)

                # vp = x M2  [1024 keys, 768] + ones columns at 768/769
                vp = vpp.tile([P, NK, VPW], BF16, tag="vp", name="vp")
                for j in range(NK):
                    nc.scalar.copy(vp[:, j, C:C + 2],
                                   ones_f[:, 2 * j:2 * j + 2])
                for r in range(NK):
                    for cs in range(2):
                        ps = mmp.tile([P, NQS], F32, tag="mm", name="ps_v")
                        for i in range(ck1):
                            nc.tensor.matmul(
                                ps[:, :CS],
                                xt[:, i, r * P:(r + 1) * P],
                                m2[:, i, cs * CS:(cs + 1) * CS],
                                start=(i == 0), stop=(i == ck1 - 1))
                        nc.scalar.copy(vp[:, r, cs * CS:(cs + 1) * CS],
                                       ps[:, :CS])

                for s in range(NSL):
                    nq0 = s * NQS
                    # S^T chunk [128 keys, 512 queries] = x T1, then exp
                    pt = ptp.tile([P, NK, NQS], BF16, tag="pt", name="pt")
                    for j in range(NK):
                        ps = mmp.tile([P, NQS], F32, tag="mm", name="ps_s")
                        for i in range(ck1):
                            nc.tensor.matmul(
                                ps[:],
                                xt[:, i, j * P:(j + 1) * P],
                                t1[:, i, nq0:nq0 + NQS],
                                start=(i == 0), stop=(i == ck1 - 1))
                        nc.scalar.activation(pt[:, j, :], ps[:], EXP,
                                             scale=SCALE)
                    # out block [128 rows, 384 cols] = P vp ; vp's ones columns
                    # ride along in the cs=1 group as the softmax denominator
                    for mi in range(NQS // P):
                        ps1 = mmp.tile([P, NQS], F32, tag="mm", name="ps_o1")
                        for j in range(NK):
                            nc.tensor.matmul(
                                ps1[:, :CS + 2],
                                pt[:, j, mi * P:(mi + 1) * P],
                                vp[:, j, CS:C + 2],
                                start=(j == 0), stop=(j == NK - 1))
                        rc = rcp.tile([P, 1], F32, tag="rc", name="rc")
                        nc.vector.reciprocal(rc[:], ps1[:, CS:CS + 1])
                        ps0 = mmp.tile([P, NQS], F32, tag="mm", name="ps_o0")
                        for j in range(NK):
                            nc.tensor.matmul(
                                ps0[:, :CS],
                                pt[:, j, mi * P:(mi + 1) * P],
                                vp[:, j, 0:CS],
                                start=(j == 0), stop=(j == NK - 1))
                        ob1 = obp.tile([P, CS], F32, tag="ob", name="ob1")
                        nc.vector.tensor_scalar_mul(ob1[:], ps1[:, :CS],
                                                    rc[:, 0:1])
                        nc.sync.dma_start(
                            out_d[b, nq0 + mi * P:nq0 + (mi + 1) * P,
                                  CS:2 * CS], ob1[:])
                        ob0 = obp.tile([P, CS], F32, tag="ob", name="ob0")
                        nc.vector.tensor_scalar_mul(ob0[:], ps0[:, :CS],
                                                    rc[:, 0:1])
                        nc.sync.dma_start(
                            out_d[b, nq0 + mi * P:nq0 + (mi + 1) * P,
                                  0:CS], ob0[:])
    nc.compile()
    return nc


def _get_program(cin):
    if cin not in _CACHE:
        _CACHE[cin] = _build_program(cin)
    return _CACHE[cin]


def _host_reference(x, W_qkv, b_qkv, W_proj, b_proj):
    out = np.empty((B, SEQ, C), dtype=np.float32)
    for b in range(B):
        qkv = x[b] @ W_qkv + b_qkv
        q, k, v = qkv[:, :C], qkv[:, C:2 * C], qkv[:, 2 * C:]
        s = (q @ k.T) * SCALE
        s -= s.max(axis=-1, keepdims=True)
        np.exp(s, out=s)
        s /= s.sum(axis=-1, keepdims=True)
        out[b] = (s @ v) @ W_proj + b_proj
    return out


def run_sharded(x, W_qkv, b_qkv, b_proj, W_proj, trace=False):
    import ml_dtypes
    from concourse.bass_utils import run_bass_kernel_spmd

    x = np.ascontiguousarray(x, dtype=np.float32)
    W_qkv = np.ascontiguousarray(W_qkv, dtype=np.float32)
    W_proj = np.ascontiguousarray(W_proj, dtype=np.float32)
    b_qkv = np.asarray(b_qkv, dtype=np.float32)
    b_proj = np.asarray(b_proj, dtype=np.float32)

    if np.any(b_qkv):
        # Cannot occur for the reference's setup_inputs (b_qkv is zeros);
        # the W_q W_k^T folding assumes zero qkv bias.
        return _host_reference(x, W_qkv, b_qkv, W_proj, b_proj), None

    bf16 = ml_dtypes.bfloat16
    M1 = np.ascontiguousarray(
        (W_qkv[:, :C] @ W_qkv[:, C:2 * C].T).astype(bf16))
    M2 = np.ascontiguousarray((W_qkv[:, 2 * C:] @ W_proj).astype(bf16))
    xT = np.ascontiguousarray(x.transpose(0, 2, 1).astype(bf16))  # [B,C,SEQ]
    nc = _get_program(C)
    in_maps = [
        {"xT": xT[c * BL:(c + 1) * BL], "m1": M1, "m2": M2}
        for c in range(NCORES)
    ]
    res = run_bass_kernel_spmd(nc, in_maps, core_ids=list(range(NCORES)),
                               trace=trace)
    out = np.concatenate([res.results[c]["out"] for c in range(NCORES)],
                         axis=0)
    out = out + b_proj[None, None, :]
    return out.astype(np.float32), res


def kernel(x, W_qkv, b_qkv, W_proj, b_proj):
    out, _ = run_sharded(x, W_qkv, b_qkv, b_proj, W_proj, trace=False)
    return out
